# revision 22
# baseline (speedup 1.0000x reference)
"""AdaptiveAttention Bass/Tile kernel for 8 trn2 NeuronCores.

Sharding: data-parallel over batch B=8, one batch element per core; the
small weights and interpolated pos table are replicated per core.

Per-core computation (N=4096 tokens, C=512, H=8 heads, D=64):
  xp   = x + pos                         (DVE add, bf16 out)
  xT   = transpose(xp)                   (DMA xbar transpose, 128x128 blocks)
  qkv  = xp @ w_qkv + b_qkv              (PE, lhsT=xT blocks, K=128 x4)
  S    = per-token head-gram  q_i . k_j  (DVE broadcast-mul + d-add-tree)
  A    = softmax_j(S/8)                  (ACT exp, DVE rowsum+recip+mul)
  O    = A @ v  per token                (DVE broadcast-mul + j-add-tree)
  OT   = transpose(O)                    (DMA xbar transpose)
  out[512h+q, :] = sum_{c1,d} O[8q+c1, h, d] * w_proj[64c1+d, :] + b_proj
       (PE, lhsT = strided columns of OT, rhs = parity-zero-padded w_proj)

The torch-faithful "scrambled" reshape (flatten of [H, N, D]) is folded
into the projection matmul's access patterns - no data movement for it.

Layout tricks (host-side, free):
  * w_qkv's V columns are permuted (j,d)->(d,j) so the AV product's
    innermost axis is j (contiguous for both operands -> DVE 2x mode).
  * w_proj is stored twice, zero-padded per head-parity, so every matmul
    operand sits at partition base 0.
"""

import sys
import numpy as np

for _p in ("/opt/trn_rl_repo", "/root/.axon_site/_ro/trn_rl_repo"):
    if _p not in sys.path:
        sys.path.insert(0, _p)

B, N, C = 8, 4096, 512
NUM_HEADS = 8
HEAD_DIM = C // NUM_HEADS
NT = N // 128            # 32 token tiles
CHUNK_TILES = 8          # token tiles per proj chunk (1024 tokens)
NCHUNK = NT // CHUNK_TILES

_CACHE = {}

# Pool buffer counts (tuned via TimelineSim cost model).
TUNE = {
    "xpp": 3, "xtp": 3, "qkvp": 2, "pbuf": 2, "tbuf": 2, "sbuf_s": 2,
    "obuf": 2, "otbuf": 2, "outsb": 3, "psq": 2, "psp": 2,
}


def _interp_linear_np(pos, out_len):
    # F.interpolate(mode='linear', align_corners=False) along axis 1.
    in_len = pos.shape[1]
    if in_len == out_len:
        return pos
    scale = in_len / out_len
    coords = (np.arange(out_len, dtype=np.float64) + 0.5) * scale - 0.5
    coords = np.clip(coords, 0.0, in_len - 1)
    i0 = np.floor(coords).astype(np.int64)
    i1 = np.minimum(i0 + 1, in_len - 1)
    w = (coords - i0).astype(np.float32)[None, :, None]
    return pos[:, i0, :] * (1.0 - w) + pos[:, i1, :] * w


def emit_core_kernel(tc, ctx, xp_d, wq_d, bq_d, wpz_d, bp_d, out_d):
    """Emit the single-core Tile program. All *_d are DRAM APs/handles."""
    import concourse.bass as bass
    from concourse import mybir

    nc = tc.nc
    f32 = mybir.dt.float32
    bf16 = mybir.dt.bfloat16
    f16 = mybir.dt.float16
    AX = mybir.AxisListType
    ALU = mybir.AluOpType
    ACTF = mybir.ActivationFunctionType

    H, D8 = NUM_HEADS, HEAD_DIM

    consts = ctx.enter_context(tc.tile_pool(name="consts", bufs=1))
    wq_sb = consts.tile([128, 4, 3 * C], bf16)       # k-chunk r = c rows 128r..
    wpz_sb = consts.tile([128, 16, C], bf16)         # [par*8 + c1] zero-padded
    bq_sb = consts.tile([1, 3 * C], bf16)
    bp_sb = consts.tile([1, C], bf16)
    ones_sb = consts.tile([1, 128], bf16)

    nc.gpsimd.dma_start(wq_sb[:], wq_d[:].rearrange("(r p) c -> p r c", p=128))
    nc.gpsimd.dma_start(wpz_sb[:], wpz_d[:].rearrange("k p c -> p k c"))
    nc.gpsimd.dma_start(bq_sb[:], bq_d[:])
    nc.gpsimd.dma_start(bp_sb[:], bp_d[:])
    nc.any.memset(ones_sb[:], 1.0)

    t = TUNE
    xpp = ctx.enter_context(tc.tile_pool(name="xpp", bufs=t["xpp"]))
    xtp = ctx.enter_context(tc.tile_pool(name="xtp", bufs=t["xtp"]))
    qkvp = ctx.enter_context(tc.tile_pool(name="qkvp", bufs=t["qkvp"]))
    pp_ = ctx.enter_context(tc.tile_pool(name="pbuf", bufs=t["pbuf"]))
    tp_ = ctx.enter_context(tc.tile_pool(name="tbuf", bufs=t["tbuf"]))
    sp_ = ctx.enter_context(tc.tile_pool(name="sbuf_s", bufs=t["sbuf_s"]))
    op_ = ctx.enter_context(tc.tile_pool(name="obuf", bufs=t["obuf"]))
    otp = ctx.enter_context(tc.tile_pool(name="otbuf", bufs=t["otbuf"]))
    osp = ctx.enter_context(tc.tile_pool(name="outsb", bufs=t["outsb"]))
    psq = ctx.enter_context(tc.tile_pool(name="psq", bufs=t["psq"], space="PSUM"))
    psp = ctx.enter_context(tc.tile_pool(name="psp", bufs=t["psp"], space="PSUM"))

    with nc.allow_low_precision(reason="bf16/fp16 attention partials; tol 2e-2"):
        for ch in range(NCHUNK):
            ot = otp.tile([128, 4, CHUNK_TILES * 128], bf16)  # [hpair, tokens]
            for t8 in range(CHUNK_TILES):
                ti = ch * CHUNK_TILES + t8
                r0 = ti * 128

                xp = xpp.tile([128, C], bf16)
                nc.gpsimd.dma_start(xp[:], xp_d[r0 : r0 + 128, :])

                xT = xtp.tile([128, 4, 128], bf16)
                nc.sync.dma_start_transpose(xT[:], xp[:])

                ps = psq.tile([128, 3 * C], f32)
                for k in range(4):
                    for nb in range(3):
                        nc.tensor.matmul(
                            ps[:, 512 * nb : 512 * (nb + 1)],
                            lhsT=xT[:, k, :],
                            rhs=wq_sb[:, k, 512 * nb : 512 * (nb + 1)],
                            start=(k == 0),
                            stop=False,
                        )
                for nb in range(3):
                    nc.tensor.matmul(
                        ps[:, 512 * nb : 512 * (nb + 1)],
                        lhsT=ones_sb[:1, :],
                        rhs=bq_sb[:1, 512 * nb : 512 * (nb + 1)],
                        start=False,
                        stop=True,
                    )

                qk = qkvp.tile([128, 3 * C], bf16)
                nc.scalar.copy(qk[:], ps[:])

                # ---- S[n, i, j] = sum_d q[n,i,d] * k[n,j,d]  (DVE) ----
                q4 = qk[:, 0:512].rearrange("p (i d) -> p i d", i=H)
                k4 = qk[:, 512:1024].rearrange("p (j d) -> p j d", j=H)
                P = pp_.tile([128, H * H, D8], f16, tag="P")
                nc.vector.tensor_mul(
                    P[:].rearrange("p (i j) d -> p i j d", i=H),
                    q4.unsqueeze(2).broadcast_to([128, H, H, D8]),
                    k4.unsqueeze(1).broadcast_to([128, H, H, D8]),
                )
                t_prev = P
                width = D8
                while width > 2:
                    half = width // 2
                    t_next = tp_.tile([128, H * H, half], f16, tag=f"T{half}")
                    nc.vector.tensor_add(
                        t_next[:], t_prev[:, :, 0:half], t_prev[:, :, half:width]
                    )
                    t_prev = t_next
                    width = half
                S = sp_.tile([128, H * H], f32, tag="S")
                nc.vector.tensor_add(S[:], t_prev[:, :, 0], t_prev[:, :, 1])

                # ---- softmax over j (scale 1/sqrt(D)) ----
                E = sp_.tile([128, H * H], bf16, tag="E")
                nc.scalar.activation(E[:], S[:], ACTF.Exp, scale=0.125)
                R = sp_.tile([128, H], f32, tag="R")
                nc.vector.tensor_reduce(
                    R[:], E[:].rearrange("p (i j) -> p i j", i=H),
                    axis=AX.X, op=ALU.add,
                )
                Rinv = sp_.tile([128, H], f32, tag="Rinv")
                nc.vector.reciprocal(Rinv[:], R[:])
                A = sp_.tile([128, H, H], bf16, tag="A")
                nc.vector.tensor_mul(
                    A[:],
                    E[:].rearrange("p (i j) -> p i j", i=H),
                    Rinv[:].unsqueeze(2).broadcast_to([128, H, H]),
                )

                # ---- O[n, i, d] = sum_j A[n,i,j] * v[n,d,j]  (DVE) ----
                v4 = qk[:, 1024:1536].rearrange("p (d j) -> p d j", d=D8)
                P2 = pp_.tile([128, H, D8, H], f16, tag="P2")
                nc.vector.tensor_mul(
                    P2[:],
                    A[:].unsqueeze(2).broadcast_to([128, H, D8, H]),
                    v4.unsqueeze(1).broadcast_to([128, H, D8, H]),
                )
                U1 = tp_.tile([128, H, D8, 4], f16, tag="U1")
                nc.vector.tensor_add(U1[:], P2[:, :, :, 0:4], P2[:, :, :, 4:8])
                U2 = tp_.tile([128, H, D8, 2], f16, tag="U2")
                nc.vector.tensor_add(U2[:], U1[:, :, :, 0:2], U1[:, :, :, 2:4])
                O = op_.tile([128, C], bf16)
                nc.vector.tensor_add(
                    O[:].rearrange("p (i d) -> p i d", i=H),
                    U2[:, :, :, 0],
                    U2[:, :, :, 1],
                )

                # ---- transpose O into the chunk's OT buffer ----
                nc.sync.dma_start_transpose(
                    ot[:, :, 128 * t8 : 128 * (t8 + 1)], O[:]
                )

            # ---- projection for this 1024-token chunk ----
            for h in range(H):
                lhs_src = ot[:, h // 2, :].rearrange("p (m s) -> p m s", s=8)
                pj = psp.tile([128, C], f32)
                for c1 in range(8):
                    nc.tensor.matmul(
                        pj[:],
                        lhsT=lhs_src[:, :, c1],
                        rhs=wpz_sb[:, (h % 2) * 8 + c1, :],
                        start=(c1 == 0),
                        stop=False,
                    )
                nc.tensor.matmul(
                    pj[:], lhsT=ones_sb[:1, :], rhs=bp_sb[:1, :],
                    start=False, stop=True,
                )
                os = osp.tile([128, C], f32)
                nc.scalar.copy(os[:], pj[:])
                ro = 512 * h + 128 * ch
                nc.gpsimd.dma_start(out_d[ro : ro + 128, :], os[:])


def build_nc():
    import concourse.tile as tile
    from concourse import bacc, mybir
    from contextlib import ExitStack

    f32 = mybir.dt.float32
    bf16 = mybir.dt.bfloat16

    nc = bacc.Bacc()
    xp_d = nc.dram_tensor("xp", [N, C], bf16, kind="ExternalInput")
    wq_d = nc.dram_tensor("wq", [C, 3 * C], bf16, kind="ExternalInput")
    bq_d = nc.dram_tensor("bq", [1, 3 * C], bf16, kind="ExternalInput")
    wpz_d = nc.dram_tensor("wpz", [16, 128, C], bf16, kind="ExternalInput")
    bp_d = nc.dram_tensor("bp", [1, C], bf16, kind="ExternalInput")
    out_d = nc.dram_tensor("out", [N, C], f32, kind="ExternalOutput")

    with tile.TileContext(nc) as tc, ExitStack() as ctx:
        emit_core_kernel(
            tc, ctx, xp_d[:], wq_d[:], bq_d[:], wpz_d[:], bp_d[:], out_d[:]
        )
    nc.compile()
    return nc


def host_prep(pos_32, w_qkv, b_qkv, w_proj, b_proj, resolution):
    """Interp pos + build the permuted/padded bf16 weight tensors."""
    import ml_dtypes

    bf = ml_dtypes.bfloat16
    target_len = int(resolution) ** 3
    pos = _interp_linear_np(np.asarray(pos_32, np.float32), target_len)
    if pos.shape[1] == N:
        pos2d = np.ascontiguousarray(pos[0], np.float32)
    else:
        # Reference only adds pos when its length matches the sequence.
        pos2d = np.zeros((N, C), np.float32)

    w_qkv = np.asarray(w_qkv, np.float32)
    b_qkv = np.asarray(b_qkv, np.float32)
    w_proj = np.asarray(w_proj, np.float32)
    b_proj = np.asarray(b_proj, np.float32)

    # Permute V columns (j, d) -> (d, j)
    wv = w_qkv[:, 1024:].reshape(C, NUM_HEADS, HEAD_DIM).transpose(0, 2, 1)
    wq = np.concatenate([w_qkv[:, :1024], wv.reshape(C, C)], axis=1).astype(bf)
    bv = b_qkv[1024:].reshape(NUM_HEADS, HEAD_DIM).T
    bq = np.concatenate([b_qkv[:1024], bv.reshape(C)])[None, :].astype(bf)

    # w_proj zero-padded per head parity: wpz[par*8+c1, par*64+d, :] = wp[64c1+d, :]
    wpz = np.zeros((16, 128, C), np.float32)
    for par in range(2):
        for c1 in range(8):
            wpz[par * 8 + c1, par * 64 : par * 64 + 64, :] = w_proj[
                64 * c1 : 64 * c1 + 64, :
            ]
    wpz = wpz.astype(bf)
    bp = b_proj[None, :].astype(bf)
    return {"pos2d": pos2d, "wq": wq, "bq": bq, "wpz": wpz, "bp": bp}


def get_runner():
    """Build the Bass program once and return a cached 8-core runner."""
    if "runner" in _CACHE:
        return _CACHE["runner"]

    import jax
    from concourse import bass2jax, mybir

    nc = build_nc()
    bass2jax.install_neuronx_cc_hook()

    part_name = nc.partition_id_tensor.name if nc.partition_id_tensor else None
    in_names, out_names, out_avals, zero_outs = [], [], [], []
    for alloc in nc.m.functions[0].allocations:
        if not isinstance(alloc, mybir.MemoryLocationSet):
            continue
        name = alloc.memorylocations[0].name
        if alloc.kind == "ExternalInput":
            if name != part_name:
                in_names.append(name)
        elif alloc.kind == "ExternalOutput":
            shape = tuple(alloc.tensor_shape)
            np_dt = mybir.dt.np(alloc.dtype)
            out_avals.append(jax.core.ShapedArray(shape, np_dt))
            out_names.append(name)
            zero_outs.append(np.zeros(shape, np_dt))
    n_params = len(in_names)
    all_names = in_names + out_names
    if part_name is not None:
        all_names = all_names + [part_name]

    def _body(*args):
        operands = list(args)
        if part_name is not None:
            operands.append(bass2jax.partition_id_tensor())
        outs = bass2jax._bass_exec_p.bind(
            *operands,
            out_avals=tuple(out_avals),
            in_names=tuple(all_names),
            out_names=tuple(out_names),
            lowering_input_output_aliases=(),
            sim_require_finite=True,
            sim_require_nnan=True,
            nc=nc,
        )
        return tuple(outs)

    devices = jax.devices()[:B]
    mesh = bass2jax.Mesh(np.asarray(devices), ("core",))
    pspec = bass2jax.PartitionSpec("core")
    sharded = jax.jit(
        bass2jax.shard_map(
            _body,
            mesh=mesh,
            in_specs=(pspec,) * (n_params + len(out_names)),
            out_specs=(pspec,) * len(out_names),
            check_rep=False,
        ),
        keep_unused=True,
    )

    runner = {
        "fn": sharded,
        "in_names": in_names,
        "out_names": out_names,
        "zero_outs": zero_outs,
        "mesh": mesh,
        "pspec": pspec,
    }
    _CACHE["runner"] = runner
    return runner


def stage_inputs(runner, per_core_maps):
    """Concatenate per-core inputs along axis 0 and place them on devices."""
    import jax
    from jax.sharding import NamedSharding

    sh = NamedSharding(runner["mesh"], runner["pspec"])
    args = []
    for name in runner["in_names"]:
        cat = np.concatenate(
            [np.asarray(m[name]) for m in per_core_maps], axis=0
        )
        args.append(jax.device_put(cat, sh))
    for z in runner["zero_outs"]:
        cat = np.zeros((B * z.shape[0], *z.shape[1:]), z.dtype)
        args.append(jax.device_put(cat, sh))
    return args


def run_staged(runner, args):
    outs = runner["fn"](*args)
    import jax

    jax.block_until_ready(outs)
    return outs


def make_per_core(x, prep):
    """Per-core input maps: fold pos into x on host (bf16)."""
    import ml_dtypes

    bf = ml_dtypes.bfloat16
    x = np.asarray(x, np.float32)
    pos2d = prep["pos2d"]
    weights = {k: v for k, v in prep.items() if k != "pos2d"}
    return [
        {"xp": (x[b] + pos2d).astype(bf), **weights} for b in range(B)
    ]


def kernel(x, pos_32, w_qkv, b_qkv, w_proj, b_proj, resolution):
    prep = host_prep(pos_32, w_qkv, b_qkv, w_proj, b_proj, resolution)

    runner = get_runner()
    per_core = make_per_core(x, prep)
    args = stage_inputs(runner, per_core)
    outs = run_staged(runner, args)
    out = np.asarray(outs[0]).reshape(B, N, C).astype(np.float32)
    return out
